# revision 5
# baseline (speedup 1.0000x reference)
"""Multi-head causal self-attention (B=2, S=2048, D=1024, H=16) on 8 NeuronCores.

Sharding: core c handles batch b = c // 4 and heads 4j..4j+3 where j = c % 4
(tensor-parallel over heads within a 4-core group, data-parallel over batch
across the two groups).  Each core:
  1. loads x[b]^T and its column slices of Wq/Wk/Wv,
  2. computes Q^T/K^T (feature-major) and V (seq-major) for its 4 heads,
  3. runs causal softmax attention per head entirely on-chip
     (scores are computed transposed, S^T[k, q], so no transposes are needed;
      the softmax denominator comes from an appended ones-column in V),
  4. AllGathers the normalized ctx^T across its 4-core group,
  5. computes its 256-column slice of the output projection (+bias).
The host assembles the 8 disjoint output slices.
"""

import math

import numpy as np

import concourse.bass as bass
import concourse.tile as tile
from concourse import bacc, mybir
from concourse.bass_utils import run_bass_kernel_spmd

B, S, D, H, DH = 2, 2048, 1024, 16, 64
NCORES = 8
GROUP = 4          # cores per batch group
HPC = 4            # heads per core
FPC = HPC * DH     # 256 features per core
QB = 512           # q block width (matmul moving free dim)
KT = 128           # k tile height (partition dim)
SCALE = 1.0 / math.sqrt(S)

F32 = mybir.dt.float32
F32R = mybir.dt.float32r


def build_program():
    nc = bacc.Bacc(
        "TRN2",
        target_bir_lowering=False,
        debug=False,
        num_devices=NCORES,
    )

    xT = nc.dram_tensor("xT", [D, S], F32R, kind="ExternalInput").ap()
    wq = nc.dram_tensor("wq", [D, FPC], F32R, kind="ExternalInput").ap()
    wk = nc.dram_tensor("wk", [D, FPC], F32R, kind="ExternalInput").ap()
    wv = nc.dram_tensor("wv", [D, FPC], F32R, kind="ExternalInput").ap()
    wo = nc.dram_tensor("wo", [D, FPC], F32R, kind="ExternalInput").ap()
    bo = nc.dram_tensor("bo", [1, FPC], F32, kind="ExternalInput").ap()
    masks = nc.dram_tensor("masks", [4, KT, QB], F32, kind="ExternalInput").ap()
    ones = nc.dram_tensor("ones", [128, 16 * HPC], F32R, kind="ExternalInput").ap()
    out = nc.dram_tensor("out", [S, FPC], F32, kind="ExternalOutput").ap()

    with tile.TileContext(nc) as tc:
        with (
            tc.tile_pool(name="cpool", bufs=1) as cpool,
            tc.tile_pool(name="qkvp", bufs=1) as qkvp,
            tc.tile_pool(name="dpool", bufs=1, space="DRAM") as dpool,
        ):
            # ---- constants / weights -------------------------------------
            wq_sb = cpool.tile([128, 8, FPC], F32R)
            wk_sb = cpool.tile([128, 8, FPC], F32R)
            wv_sb = cpool.tile([128, 8, FPC], F32R)
            wo_sb = cpool.tile([128, 8, FPC], F32R)
            for w_sb, w in ((wq_sb, wq), (wk_sb, wk), (wv_sb, wv), (wo_sb, wo)):
                nc.sync.dma_start(w_sb[:], w.rearrange("(t p) f -> p t f", p=128))
            bo_sb = cpool.tile([1, FPC], F32)
            nc.sync.dma_start(bo_sb[:], bo)
            bias_bc = cpool.tile([128, FPC], F32)
            nc.gpsimd.partition_broadcast(bias_bc[:], bo_sb[:])
            mask_sb = cpool.tile([128, 4, QB], F32)
            nc.sync.dma_start(mask_sb[:], masks.rearrange("o p q -> p o q"))

            # ---- persistent per-core tensors -----------------------------
            qT_sb = qkvp.tile([128, 2, S], F32R)   # [dh, head-pair, seq]
            kT_sb = qkvp.tile([128, 2, S], F32R)
            v_sb = qkvp.tile([128, 16, HPC * (DH + 1)], F32R)  # [k, seqtile, 4*(64+1)]
            v4 = v_sb.rearrange("p s (h e) -> p s h e", e=DH + 1)

            cc_in = dpool.tile([FPC, S], F32R)
            cc_out = dpool.tile([GROUP * FPC, S], F32R)

            # ---- projections ---------------------------------------------
            with (
                tc.tile_pool(name="xtp", bufs=1) as xtp,
                tc.tile_pool(name="pjp", bufs=3, space="PSUM") as pjp,
            ):
                xt_sb = xtp.tile([128, 8, S], F32R)
                xt_dram = xT.rearrange("(t p) m -> t p m", p=128)
                for t in range(8):
                    nc.sync.dma_start(xt_sb[:, t], xt_dram[t])

                # ones column for the softmax-denominator trick
                nc.sync.dma_start(
                    v4[:, :, :, DH], ones.rearrange("p (s h) -> p s h", h=HPC)
                )

                # Q^T / K^T: feature-major, heads packed 2-per-128-partitions
                for w_sb, dst in ((wq_sb, qT_sb), (wk_sb, kT_sb)):
                    for f in range(2):
                        for qb in range(4):
                            ps = pjp.tile([128, QB], F32, tag="pj")
                            for t in range(8):
                                nc.tensor.matmul(
                                    ps[:],
                                    w_sb[:, t, f * 128:(f + 1) * 128],
                                    xt_sb[:, t, qb * QB:(qb + 1) * QB],
                                    start=(t == 0),
                                    stop=(t == 7),
                                )
                            nc.scalar.copy(dst[:, f, qb * QB:(qb + 1) * QB], ps[:])

                # V: seq-major, heads interleaved with ones column
                for s in range(16):
                    ps = pjp.tile([128, FPC], F32, tag="pjv")
                    for t in range(8):
                        nc.tensor.matmul(
                            ps[:],
                            xt_sb[:, t, s * 128:(s + 1) * 128],
                            wv_sb[:, t],
                            start=(t == 0),
                            stop=(t == 7),
                        )
                    nc.vector.tensor_copy(
                        v4[:, s, :, 0:DH],
                        ps.rearrange("p (h e) -> p h e", e=DH),
                    )

            # ---- attention ------------------------------------------------
            with (
                tc.tile_pool(name="attp", bufs=4) as attp,
                tc.tile_pool(name="stp", bufs=4, space="PSUM") as stp,
                tc.tile_pool(name="ctxp", bufs=3, space="PSUM") as ctxp,
                tc.tile_pool(name="nrmp", bufs=3) as nrmp,
            ):
                for pair in range(2):
                    h0, h1 = 2 * pair, 2 * pair + 1
                    for qb in range(4):
                        nk = 4 * (qb + 1)
                        qs = slice(qb * QB, (qb + 1) * QB)
                        ctx0 = ctxp.tile([128, QB], F32, tag="ctx")
                        ctx1 = ctxp.tile([128, QB], F32, tag="ctx")
                        for ki in range(nk):
                            ks = slice(ki * KT, (ki + 1) * KT)
                            st0 = stp.tile([128, QB], F32, tag="st")
                            st1 = stp.tile([128, QB], F32, tag="st")
                            nc.tensor.matmul(
                                st0[:], kT_sb[0:64, pair, ks],
                                qT_sb[0:64, pair, qs], start=True, stop=True,
                            )
                            nc.tensor.matmul(
                                st1[:], kT_sb[64:128, pair, ks],
                                qT_sb[64:128, pair, qs], start=True, stop=True,
                            )
                            pt0 = attp.tile([128, QB], F32R, tag="pt")
                            pt1 = attp.tile([128, QB], F32R, tag="pt")
                            nc.scalar.activation(
                                pt0[:], st0[:], mybir.ActivationFunctionType.Exp,
                                scale=SCALE,
                            )
                            nc.scalar.activation(
                                pt1[:], st1[:], mybir.ActivationFunctionType.Exp,
                                scale=SCALE,
                            )
                            off = ki * KT - qb * QB
                            if off >= 0:
                                oi = off // KT
                                nc.vector.tensor_mul(pt0[:], pt0[:], mask_sb[:, oi])
                                nc.vector.tensor_mul(pt1[:], pt1[:], mask_sb[:, oi])
                            nc.tensor.matmul(
                                ctx0[0:DH + 1], v4[:, ki, h0], pt0[:],
                                start=(ki == 0), stop=(ki == nk - 1),
                            )
                            nc.tensor.matmul(
                                ctx1[0:DH + 1], v4[:, ki, h1], pt1[:],
                                start=(ki == 0), stop=(ki == nk - 1),
                            )
                        for h, ctx in ((h0, ctx0), (h1, ctx1)):
                            rc = nrmp.tile([1, QB], F32, tag="rc")
                            nc.vector.reciprocal(rc[:], ctx[DH:DH + 1, :])
                            bc = nrmp.tile([64, QB], F32, tag="bc")
                            nc.gpsimd.partition_broadcast(bc[:], rc[:])
                            cn = nrmp.tile([64, QB], F32R, tag="cn")
                            nc.vector.tensor_mul(cn[:], ctx[0:DH, :], bc[:])
                            nc.sync.dma_start(
                                cc_in[h * DH:(h + 1) * DH, qs], cn[:]
                            )

            # ---- gather ctx^T across the 4-core group ---------------------
            nc.gpsimd.collective_compute(
                "AllGather",
                mybir.AluOpType.bypass,
                replica_groups=[[0, 1, 2, 3], [4, 5, 6, 7]],
                ins=[cc_in.opt()],
                outs=[cc_out.opt()],
            )

            # ---- output projection ---------------------------------------
            with (
                tc.tile_pool(name="ogp", bufs=1) as ogp,
                tc.tile_pool(name="opp", bufs=3, space="PSUM") as opp,
                tc.tile_pool(name="obp", bufs=3) as obp,
            ):
                ctxg = ogp.tile([128, 8, S], F32R)
                ccg = cc_out.rearrange("(f p) q -> f p q", p=128)
                for f in range(8):
                    nc.sync.dma_start(ctxg[:, f], ccg[f])
                for s in range(16):
                    ps = opp.tile([128, FPC], F32, tag="op")
                    for f in range(8):
                        nc.tensor.matmul(
                            ps[:],
                            ctxg[:, f, s * 128:(s + 1) * 128],
                            wo_sb[:, f],
                            start=(f == 0),
                            stop=(f == 7),
                        )
                    ot = obp.tile([128, FPC], F32, tag="ot")
                    nc.vector.tensor_add(ot[:], ps[:], bias_bc[:])
                    nc.sync.dma_start(out[s * 128:(s + 1) * 128, :], ot[:])

    nc.compile()
    return nc


_PROGRAM = None


def _get_program():
    global _PROGRAM
    if _PROGRAM is None:
        _PROGRAM = build_program()
    return _PROGRAM


def _make_masks():
    masks = np.zeros((4, KT, QB), np.float32)
    for oi in range(4):
        off = oi * KT
        k = np.arange(KT)[:, None] + off
        q = np.arange(QB)[None, :]
        masks[oi] = (k <= q).astype(np.float32)
    return masks


def kernel(x, Wq, Wk, Wv, Wo, bo):
    x = np.ascontiguousarray(np.asarray(x, np.float32))
    Wq = np.asarray(Wq, np.float32)
    Wk = np.asarray(Wk, np.float32)
    Wv = np.asarray(Wv, np.float32)
    Wo = np.asarray(Wo, np.float32)
    bo = np.asarray(bo, np.float32)

    masks = _make_masks()
    ones_arr = np.ones((128, 16 * HPC), np.float32)
    xTs = [np.ascontiguousarray(x[b].T) for b in range(B)]

    in_maps = []
    for c in range(NCORES):
        b, j = divmod(c, GROUP)
        cols = slice(FPC * j, FPC * (j + 1))
        in_maps.append({
            "xT": xTs[b],
            "wq": np.ascontiguousarray(Wq[:, cols]),
            "wk": np.ascontiguousarray(Wk[:, cols]),
            "wv": np.ascontiguousarray(Wv[:, cols]),
            "wo": np.ascontiguousarray(Wo[:, cols]),
            "bo": np.ascontiguousarray(bo[cols][None, :]),
            "masks": masks,
            "ones": ones_arr,
        })

    nc = _get_program()
    results = run_bass_kernel_spmd(nc, in_maps, list(range(NCORES))).results

    out = np.empty((B, S, D), np.float32)
    for c in range(NCORES):
        b, j = divmod(c, GROUP)
        out[b, :, FPC * j:FPC * (j + 1)] = results[c]["out"]
    return out
